# revision 5
# baseline (speedup 1.0000x reference)
"""BitLinear forward on 8 TRN2 NeuronCores (tensor-parallel, column-parallel linear).

  alpha = mean(|W|)            (scalar over the FULL weight matrix)
  y     = x @ (sign(W) * alpha)^T

Sharding: W rows (out_features) split across 8 cores; x replicated; each core
computes y[:, c*2048:(c+1)*2048]; alpha = local |W| reduction + AllReduce.

Per-core device pipeline:
  1. W pass: load W shard fp32 (half-row tiles), sign()->bf16 scratch in DRAM,
     abs-row-sums on the side.
  2. alpha: DVE reduce -> gpsimd partition_all_reduce -> AllReduce(8 cores).
  3. WT: XBAR DMA-transpose-load sign(W) -> SBUF, cast fp8e4 (+-1 exact; mixed
     bf16 x fp8 matmul runs at bf16 rate, halves SBUF). Transposes alternate
     between the two HWDGE issue engines (sync/scalar) to halve issue latency.
  4. Per 128-row x tile: load fp32 -> cast bf16 -> DRAM scratch -> DMA-transpose
     -> xT [128, 32, 128]; 32x4 matmuls accumulate [128, 2048] fp32 in PSUM;
     ScalarE Copy*alpha eviction; DMA out.

Matmul mapping: out[s, o] += xT[i, s].T @ WT[i, o]  (K=i on partitions).

Post-schedule pass: the tile legalizer emits one LDWEIGHTS per MATMUL; the 4
matmuls per k-group share a stationary, so 3 of 4 loads are redundant (~47ns
each on the PE critical path). dedupe_ldweights() removes consecutive
LDWEIGHTS with identical weight APs and no semaphore waits/updates.
"""
import sys
import os

sys.path.insert(0, "/opt/trn_rl_repo")
import numpy as np

P = 128
S, I, O = 8192, 4096, 16384
N_CORES = 8
OC = O // N_CORES          # 2048 out-features per core
KB = I // P                # 32 contraction blocks
NT = S // P                # 64 x row-tiles
NJ = OC // 512             # 4 psum bank chunks

_cache = {}


def _ldw_sig(i):
    a = i.ins[0]
    return (
        str(getattr(a, "name", "")), str(getattr(a, "ap", "")),
        str(getattr(a, "offset", "")), str(getattr(a, "dtype", "")),
        str(i.perf_mode), str(i.is_transpose), str(i.tile_position),
    )


def dedupe_ldweights(nc):
    """Remove LDWEIGHTS that reload the stationary already in the PE array."""
    removed = 0
    for bb in nc.main_func.blocks:
        new = []
        last_sig = None
        for i in bb.instructions:
            tn = type(i).__name__
            if tn == "InstLdweights":
                sig = _ldw_sig(i)
                si = i.sync_info
                clean = si is None or (not si.on_wait and not si.on_update)
                if sig == last_sig and clean:
                    removed += 1
                    continue
                last_sig = sig
            elif tn in ("InstMatmult", "InstEventSemaphore", "InstRegisterMove",
                        "InstNop"):
                pass  # these leave the loaded stationary intact
            else:
                last_sig = None
            new.append(i)
        bb.instructions[:] = new
    return removed


def _build():
    from concourse import bacc, tile, mybir, bass_isa

    dt = mybir.dt
    nc = bacc.Bacc("TRN2", target_bir_lowering=False, debug=False, num_devices=N_CORES)
    x_ap = nc.dram_tensor("x", [S, I], dt.float32, kind="ExternalInput").ap()
    w_ap = nc.dram_tensor("w", [OC, I], dt.float32, kind="ExternalInput").ap()
    y_ap = nc.dram_tensor("y", [S, OC], dt.float32, kind="ExternalOutput").ap()

    HI = I // 2  # W pass processes half-rows

    with tile.TileContext(nc) as tc:
        with (
            tc.tile_pool(name="pers", bufs=1) as pers,
            tc.tile_pool(name="wld", bufs=3) as wld,
            tc.tile_pool(name="wsg", bufs=3) as wsg,
            tc.tile_pool(name="xld", bufs=2) as xld,
            tc.tile_pool(name="xsg", bufs=2) as xsg,
            tc.tile_pool(name="wtmp", bufs=2) as wtmp,
            tc.tile_pool(name="pxT", bufs=3) as pxT,
            tc.tile_pool(name="pyo", bufs=2) as pyo,
            tc.tile_pool(name="psum", bufs=2, space="PSUM") as psum,
            tc.tile_pool(name="dramw", bufs=1, space="DRAM") as dramw,
            tc.tile_pool(name="dramx", bufs=NT, space="DRAM") as dramx,
            tc.tile_pool(name="dramc", bufs=1, space="DRAM") as dramc,
        ):
            # ---- W pass: sign -> bf16 scratch, |w| partial sums
            wsgn = dramw.tile([OC, I], dt.bfloat16)
            wabs = pers.tile([P, 2 * (OC // P)], dt.float32)
            for t in range(OC // P):
                for h in range(2):
                    w32 = wld.tile([P, HI], dt.float32, tag="wld")
                    nc.sync.dma_start(w32[:], w_ap[t * P:(t + 1) * P, h * HI:(h + 1) * HI])
                    sg = wsg.tile([P, HI], dt.bfloat16, tag="wsg")
                    nc.scalar.sign(sg[:], w32[:])
                    nc.sync.dma_start(wsgn[t * P:(t + 1) * P, h * HI:(h + 1) * HI], sg[:])
                    nc.vector.tensor_reduce(
                        wabs[:, 2 * t + h:2 * t + h + 1], w32[:],
                        axis=mybir.AxisListType.XYZW,
                        op=mybir.AluOpType.add, apply_absolute_value=True)

            # ---- alpha: local reduce -> partition allreduce -> 8-core AllReduce
            wsum = pers.tile([P, 1], dt.float32)
            nc.vector.tensor_reduce(
                wsum[:], wabs[:], axis=mybir.AxisListType.XYZW,
                op=mybir.AluOpType.add)
            par = pers.tile([P, 1], dt.float32)
            nc.gpsimd.partition_all_reduce(
                par[:], wsum[:], channels=P, reduce_op=bass_isa.ReduceOp.add)
            cc_in = dramc.tile([P, 1], dt.float32)
            cc_out = dramc.tile([P, 1], dt.float32, addr_space="Shared")
            nc.sync.dma_start(cc_in[:], par[:])
            nc.gpsimd.collective_compute(
                "AllReduce", mybir.AluOpType.add,
                replica_groups=[list(range(N_CORES))],
                ins=[cc_in[:].opt()], outs=[cc_out[:].opt()])
            asum = pers.tile([P, 1], dt.float32)
            nc.sync.dma_start(asum[:], cc_out[:])
            alpha = pers.tile([P, 1], dt.float32)
            nc.vector.tensor_scalar_mul(alpha[:], asum[:], 1.0 / (float(O) * float(I)))

            # ---- WT: transpose-load sign(W), cast to fp8; alternate issue engines
            WT = pers.tile([P, KB, OC], dt.float8e4)
            for c in range(16):
                tmp = wtmp.tile([P, 2, OC], dt.bfloat16, tag="wtmp")
                nc.sync.dma_start_transpose(tmp[:], wsgn[:, c * 256:(c + 1) * 256])
                nc.vector.tensor_copy(WT[:, 2 * c:2 * c + 2, :], tmp[:])

            # ---- main loop over x row-tiles
            for st in range(NT):
                x32 = xld.tile([P, I], dt.float32, tag="xld")
                nc.sync.dma_start(x32[:], x_ap[st * P:(st + 1) * P, :])
                xc = xsg.tile([P, I], dt.bfloat16, tag="xsg")
                nc.vector.tensor_copy(xc[:], x32[:])
                xb = dramx.tile([P, I], dt.bfloat16, tag="xb")
                nc.sync.dma_start(xb[:], xc[:])
                xT = pxT.tile([P, KB, P], dt.bfloat16, tag="xT")
                nc.sync.dma_start_transpose(xT[:], xb[:])
                ps = psum.tile([P, OC], dt.float32, tag="ps")
                for k in range(KB):
                    for j in range(NJ):
                        nc.tensor.matmul(
                            ps[:, j * 512:(j + 1) * 512],
                            xT[:, k, :],
                            WT[:, k, j * 512:(j + 1) * 512],
                            start=(k == 0), stop=(k == KB - 1))
                yo = pyo.tile([P, OC], dt.float32, tag="yo")
                nc.scalar.activation(
                    yo[:], ps[:], mybir.ActivationFunctionType.Copy,
                    bias=0.0, scale=alpha[:, 0:1])
                nc.sync.dma_start(y_ap[st * P:(st + 1) * P, :], yo[:])

    # NOTE: dedupe_ldweights() (removing per-MATMUL weight reloads) was tried
    # and produces wrong results on HW — the PE's weight-buffer management
    # assumes each MATMUL self-loads. Do not re-enable without a fix.
    nc.compile()
    return nc


def _get_nc():
    if "nc" not in _cache:
        _cache["nc"] = _build()
    return _cache["nc"]


def kernel(x: np.ndarray, weight: np.ndarray) -> np.ndarray:
    from concourse.bass_utils import run_bass_kernel_spmd

    nc = _get_nc()
    xf = np.ascontiguousarray(np.asarray(x, dtype=np.float32).reshape(S, I))
    wf = np.asarray(weight, dtype=np.float32)
    in_maps = [
        {"x": xf, "w": np.ascontiguousarray(wf[c * OC:(c + 1) * OC])}
        for c in range(N_CORES)
    ]
    res = run_bass_kernel_spmd(
        nc, in_maps, core_ids=list(range(N_CORES)),
        trace=bool(int(os.environ.get("BITLINEAR_TRACE", "0"))),
    )
    if res.exec_time_ns is not None:
        _cache["exec_time_ns"] = res.exec_time_ns
    _cache["last_results"] = res
    y = np.concatenate([res.results[c]["y"] for c in range(N_CORES)], axis=1)
    return y.reshape(2, S // 2, O)


# revision 11
# speedup vs baseline: 1.0766x; 1.0766x over previous
"""BitLinear forward on 8 TRN2 NeuronCores (tensor-parallel, column-parallel linear).

  alpha = mean(|W|)            (scalar over the FULL weight matrix)
  y     = x @ (sign(W) * alpha)^T

Sharding: W rows (out_features) split across 8 cores; x replicated; each core
computes y[:, c*2048:(c+1)*2048]; alpha = local |W| reduction + AllReduce.

Per-core device pipeline:
  1. W pass: load W shard fp32 (half-row tiles), sign()->bf16 scratch in DRAM,
     abs-row-sums on the side.
  2. alpha: DVE reduce -> gpsimd partition_all_reduce -> AllReduce(8 cores).
  3. WT: XBAR DMA-transpose-load sign(W) -> SBUF, cast fp8e4 (+-1 exact; mixed
     bf16 x fp8 matmul runs at bf16 rate, halves SBUF). Transposes alternate
     between the two HWDGE issue engines (sync/scalar) to halve issue latency.
  4. Per 128-row x tile: load fp32 -> cast bf16 -> DRAM scratch -> DMA-transpose
     -> xT [128, 32, 128]; 32x4 matmuls accumulate [128, 2048] fp32 in PSUM;
     ScalarE Copy*alpha eviction; DMA out.

Matmul mapping: out[s, o] += xT[i, s].T @ WT[i, o]  (K=i on partitions).

Post-schedule pass: the tile legalizer emits one LDWEIGHTS per MATMUL; the 4
matmuls per k-group share a stationary, so 3 of 4 loads are redundant (~47ns
each on the PE critical path). dedupe_ldweights() removes consecutive
LDWEIGHTS with identical weight APs and no semaphore waits/updates.
"""
import sys
import os

sys.path.insert(0, "/opt/trn_rl_repo")
import numpy as np

P = 128
S, I, O = 8192, 4096, 16384
N_CORES = 8
OC = O // N_CORES          # 2048 out-features per core
KB = I // P                # 32 contraction blocks
NT = S // P                # 64 x row-tiles
NJ = OC // 512             # 4 psum bank chunks

_cache = {}


def _ldw_sig(i):
    a = i.ins[0]
    return (
        str(getattr(a, "name", "")), str(getattr(a, "ap", "")),
        str(getattr(a, "offset", "")), str(getattr(a, "dtype", "")),
        str(i.perf_mode), str(i.is_transpose), str(i.tile_position),
    )


def dedupe_ldweights(nc):
    """Remove LDWEIGHTS that reload the stationary already in the PE array."""
    removed = 0
    for bb in nc.main_func.blocks:
        new = []
        last_sig = None
        for i in bb.instructions:
            tn = type(i).__name__
            if tn == "InstLdweights":
                sig = _ldw_sig(i)
                si = i.sync_info
                clean = si is None or (not si.on_wait and not si.on_update)
                if sig == last_sig and clean:
                    removed += 1
                    continue
                last_sig = sig
            elif tn in ("InstMatmult", "InstEventSemaphore", "InstRegisterMove",
                        "InstNop"):
                pass  # these leave the loaded stationary intact
            else:
                last_sig = None
            new.append(i)
        bb.instructions[:] = new
    return removed


def _build():
    from concourse import bacc, tile, mybir, bass_isa

    dt = mybir.dt
    nc = bacc.Bacc("TRN2", target_bir_lowering=False, debug=False, num_devices=N_CORES)
    x_ap = nc.dram_tensor("x", [S, I], dt.float32, kind="ExternalInput").ap()
    w_ap = nc.dram_tensor("w", [OC, I], dt.float32, kind="ExternalInput").ap()
    y_ap = nc.dram_tensor("y", [S, OC], dt.float32, kind="ExternalOutput").ap()

    from concourse.bass import _add_dep_helper

    HI = I // 2  # W pass processes half-rows

    with tile.TileContext(nc) as tc:
        with (
            tc.tile_pool(name="pers", bufs=1) as pers,
            tc.tile_pool(name="wld", bufs=3) as wld,
            tc.tile_pool(name="wsg", bufs=3) as wsg,
            tc.tile_pool(name="xld", bufs=2) as xld,
            tc.tile_pool(name="xsg", bufs=2) as xsg,
            tc.tile_pool(name="wtmp", bufs=2) as wtmp,
            tc.tile_pool(name="pxT", bufs=3) as pxT,
            tc.tile_pool(name="pyo", bufs=2) as pyo,
            tc.tile_pool(name="psum", bufs=2, space="PSUM") as psum,
            tc.tile_pool(name="dramw", bufs=1, space="DRAM") as dramw,
            tc.tile_pool(name="dramx", bufs=NT, space="DRAM") as dramx,
            tc.tile_pool(name="dramc", bufs=1, space="DRAM") as dramc,
        ):
            # ---- W pass: sign -> bf16 scratch, |w| partial sums
            wsgn = dramw.tile([OC, I], dt.bfloat16)
            wabs = pers.tile([P, 2 * (OC // P)], dt.float32)
            w_stores = []
            for t in range(OC // P):
                for h in range(2):
                    w32 = wld.tile([P, HI], dt.float32, tag="wld")
                    nc.sync.dma_start(w32[:], w_ap[t * P:(t + 1) * P, h * HI:(h + 1) * HI])
                    sg = wsg.tile([P, HI], dt.bfloat16, tag="wsg")
                    nc.scalar.sign(sg[:], w32[:])
                    w_stores.append(nc.sync.dma_start(
                        wsgn[t * P:(t + 1) * P, h * HI:(h + 1) * HI], sg[:]))
                    nc.vector.tensor_reduce(
                        wabs[:, 2 * t + h:2 * t + h + 1], w32[:],
                        axis=mybir.AxisListType.XYZW,
                        op=mybir.AluOpType.add, apply_absolute_value=True)

            # ---- alpha: local reduce -> partition allreduce -> 8-core AllReduce
            wsum = pers.tile([P, 1], dt.float32)
            nc.vector.tensor_reduce(
                wsum[:], wabs[:], axis=mybir.AxisListType.XYZW,
                op=mybir.AluOpType.add)
            par = pers.tile([P, 1], dt.float32)
            nc.gpsimd.partition_all_reduce(
                par[:], wsum[:], channels=P, reduce_op=bass_isa.ReduceOp.add)
            cc_in = dramc.tile([P, 1], dt.float32)
            cc_out = dramc.tile([P, 1], dt.float32, addr_space="Shared")
            nc.sync.dma_start(cc_in[:], par[:])
            nc.gpsimd.collective_compute(
                "AllReduce", mybir.AluOpType.add,
                replica_groups=[list(range(N_CORES))],
                ins=[cc_in[:].opt()], outs=[cc_out[:].opt()])
            asum = pers.tile([P, 1], dt.float32)
            nc.sync.dma_start(asum[:], cc_out[:])
            alpha = pers.tile([P, 1], dt.float32)
            nc.vector.tensor_scalar_mul(alpha[:], asum[:], 1.0 / (float(O) * float(I)))

            # ---- WT: transpose-load sign(W), cast to fp8; alternate issue engines
            WT = pers.tile([P, KB, OC], dt.float8e4)
            for c in range(16):
                tmp = wtmp.tile([P, 2, OC], dt.bfloat16, tag="wtmp")
                nc.sync.dma_start_transpose(tmp[:], wsgn[:, c * 256:(c + 1) * 256])
                # cast on ScalarE: VectorE is busy with reductions/x casts here
                nc.scalar.activation(WT[:, 2 * c:2 * c + 2, :], tmp[:],
                                     mybir.ActivationFunctionType.Copy)

            # ---- main loop over x row-tiles
            for st in range(NT):
                x32 = xld.tile([P, I], dt.float32, tag="xld")
                xl = nc.sync.dma_start(x32[:], x_ap[st * P:(st + 1) * P, :])
                if st < 2:
                    # keep the early x loads from stealing DMA bandwidth from
                    # the W pass, which gates the first matmul
                    _add_dep_helper(xl.ins, w_stores[-1].ins, sync=True,
                                    reason="hold x behind W preprocess")
                xc = xsg.tile([P, I], dt.bfloat16, tag="xsg")
                nc.vector.tensor_copy(xc[:], x32[:])
                xb = dramx.tile([P, I], dt.bfloat16, tag="xb")
                nc.sync.dma_start(xb[:], xc[:])
                xT = pxT.tile([P, KB, P], dt.bfloat16, tag="xT")
                nc.sync.dma_start_transpose(xT[:], xb[:])
                ps = psum.tile([P, OC], dt.float32, tag="ps")
                for k in range(KB):
                    for j in range(NJ):
                        nc.tensor.matmul(
                            ps[:, j * 512:(j + 1) * 512],
                            xT[:, k, :],
                            WT[:, k, j * 512:(j + 1) * 512],
                            start=(k == 0), stop=(k == KB - 1))
                yo = pyo.tile([P, OC], dt.float32, tag="yo")
                nc.scalar.activation(
                    yo[:], ps[:], mybir.ActivationFunctionType.Copy,
                    bias=0.0, scale=alpha[:, 0:1])
                nc.sync.dma_start(y_ap[st * P:(st + 1) * P, :], yo[:])

    # NOTE: dedupe_ldweights() (removing per-MATMUL weight reloads) was tried
    # and produces wrong results on HW — the PE's weight-buffer management
    # assumes each MATMUL self-loads. Do not re-enable without a fix.
    nc.compile()
    return nc


def _get_nc():
    if "nc" not in _cache:
        _cache["nc"] = _build()
    return _cache["nc"]


def kernel(x: np.ndarray, weight: np.ndarray) -> np.ndarray:
    from concourse.bass_utils import run_bass_kernel_spmd

    nc = _get_nc()
    xf = np.ascontiguousarray(np.asarray(x, dtype=np.float32).reshape(S, I))
    wf = np.asarray(weight, dtype=np.float32)
    in_maps = [
        {"x": xf, "w": np.ascontiguousarray(wf[c * OC:(c + 1) * OC])}
        for c in range(N_CORES)
    ]
    res = run_bass_kernel_spmd(
        nc, in_maps, core_ids=list(range(N_CORES)),
        trace=bool(int(os.environ.get("BITLINEAR_TRACE", "0"))),
    )
    if res.exec_time_ns is not None:
        _cache["exec_time_ns"] = res.exec_time_ns
    _cache["last_results"] = res
    y = np.concatenate([res.results[c]["y"] for c in range(N_CORES)], axis=1)
    return y.reshape(2, S // 2, O)
